# revision 1
# baseline (speedup 1.0000x reference)
"""Multi-head attention (b=2, n=2048, dim=1024, 16 heads x 64) on 8 TRN2 NeuronCores.

Sharding: core c handles batch c//4 and heads 4*(c%4) .. 4*(c%4)+3
(data parallel over batch x 4-way head/tensor parallel). w_qkv is
column-sharded by head; w_out is column-sharded (each core computes a
256-column slice of the output after an AllGather of all heads' attention
outputs within its 4-core batch group), so no all-reduce is needed.

Device-side layout is feature-major ("K-major"): x arrives pre-transposed
[dim, n]; Q^T/K^T are produced feature-major, V token-major (both directly
from the QKV projection, no on-chip transposes); attention scores are
computed transposed (dotsT[k, q]) so softmax sums come from an augmented
ones-column in the V matmul.
"""

import sys

sys.path.insert(0, "/opt/trn_rl_repo")

import numpy as np

import concourse.bass as bass  # noqa: F401
import concourse.tile as tile
from concourse import bacc, mybir
from concourse.bass_utils import run_bass_kernel_spmd

F32 = mybir.dt.float32
F32R = mybir.dt.float32r

# Problem constants
B, N, DIM = 2, 2048, 1024
HEADS, DH = 16, 64
INNER = HEADS * DH
SCALE = DH ** -0.5
CORES = 8
GROUP_SIZE = 4
REPLICA_GROUPS = [[0, 1, 2, 3], [4, 5, 6, 7]]
HPC = 4  # heads per core
CS = HPC * DH  # 256 per-core feature columns

KC = DIM // 128  # 8 contraction chunks for dim
TT = N // 128  # 16 token tiles
QB = N // 512  # 4 q blocks
NKC = N // 128  # 16 key chunks


def build_nc():
    nc = bacc.Bacc("TRN2", target_bir_lowering=False, debug=False, num_devices=CORES)
    xt = nc.dram_tensor("xt", [DIM, N], F32R, kind="ExternalInput").ap()
    wq = nc.dram_tensor("wq", [DIM, CS], F32R, kind="ExternalInput").ap()
    wk = nc.dram_tensor("wk", [DIM, CS], F32R, kind="ExternalInput").ap()
    wv = nc.dram_tensor("wv", [DIM, CS], F32R, kind="ExternalInput").ap()
    wo = nc.dram_tensor("wo", [INNER, CS], F32R, kind="ExternalInput").ap()
    bo = nc.dram_tensor("bo", [CS], F32, kind="ExternalInput")
    y = nc.dram_tensor("y", [N, CS], F32, kind="ExternalOutput").ap()

    cc_in = nc.dram_tensor("cc_in", [CS, N], F32R)
    cc_out = nc.dram_tensor("cc_out", [INNER, N], F32R)

    with tile.TileContext(nc) as tc:
        with (
            tc.tile_pool(name="big", bufs=1) as big,  # xt, later AG result
            tc.tile_pool(name="sb", bufs=1) as sb,
            tc.tile_pool(name="expp", bufs=4) as expp,
            tc.tile_pool(name="yout", bufs=3) as yout,
            tc.tile_pool(name="norm", bufs=2) as normp,
            tc.tile_pool(name="psd", bufs=2, space="PSUM") as psd,
            tc.tile_pool(name="pso", bufs=3, space="PSUM") as pso,
            tc.tile_pool(name="psb", bufs=1, space="PSUM") as psbp,
        ):
            # ---- load inputs -------------------------------------------------
            xt_sb = big.tile([128, KC, N], F32R, tag="bigbuf")
            xt_r = xt.rearrange("(c p) n -> p c n", p=128)
            for c in range(KC):
                nc.sync.dma_start(out=xt_sb[:, c, :], in_=xt_r[:, c, :])
            wq_sb = sb.tile([128, KC, CS], F32R)
            wk_sb = sb.tile([128, KC, CS], F32R)
            wv_sb = sb.tile([128, KC, CS], F32R)
            wo_sb = sb.tile([128, KC, CS], F32R)
            nc.sync.dma_start(out=wq_sb, in_=wq.rearrange("(c p) n -> p c n", p=128))
            nc.sync.dma_start(out=wk_sb, in_=wk.rearrange("(c p) n -> p c n", p=128))
            nc.sync.dma_start(out=wv_sb, in_=wv.rearrange("(c p) n -> p c n", p=128))
            nc.sync.dma_start(out=wo_sb, in_=wo.rearrange("(c p) n -> p c n", p=128))

            bias_sb = sb.tile([128, CS], F32)
            bo_bcast = bass.AP(tensor=bo, offset=0, ap=[[0, 128], [1, CS]])
            nc.sync.dma_start(out=bias_sb, in_=bo_bcast)

            ones_f = sb.tile([128, NKC], F32)
            nc.vector.memset(ones_f, 1.0)
            ones_r = sb.tile([1, DH], F32R)
            nc.vector.tensor_copy(ones_r, ones_f[0:1, 0:1].broadcast_to([1, DH]))

            # ---- QKV projection ---------------------------------------------
            # Q^T, K^T feature-major: [128 = head-pair x 64, N] per hp
            qt_sb = sb.tile([128, 2, N], F32R)
            kt_sb = sb.tile([128, 2, N], F32R)
            # V token-major, augmented with a ones column per head:
            # [128, token-tile, head, 65]
            vaug = sb.tile([128, TT, HPC, DH + 1], F32R)
            for h in range(HPC):
                nc.vector.tensor_copy(vaug[:, :, h, DH], ones_f)

            for m in range(2):  # feature blocks of 128 = head pairs
                for qb in range(QB):
                    for dst, w_sb in ((qt_sb, wq_sb), (kt_sb, wk_sb)):
                        ps = psd.tile([128, 2, 512], F32, name="psd")
                        acc = ps[:, 0, :]
                        for c in range(KC):
                            nc.tensor.matmul(
                                acc,
                                lhsT=w_sb[:, c, m * 128 : (m + 1) * 128],
                                rhs=xt_sb[:, c, qb * 512 : (qb + 1) * 512],
                                start=(c == 0),
                                stop=(c == KC - 1),
                            )
                        nc.vector.tensor_copy(
                            dst[:, m, qb * 512 : (qb + 1) * 512], acc
                        )
            for t in range(TT):
                ps = psd.tile([128, 2, 512], F32, name="psd")
                acc = ps[:, 0, 0:CS]
                for c in range(KC):
                    nc.tensor.matmul(
                        acc,
                        lhsT=xt_sb[:, c, t * 128 : (t + 1) * 128],
                        rhs=wv_sb[:, c, :],
                        start=(c == 0),
                        stop=(c == KC - 1),
                    )
                nc.vector.tensor_copy(
                    vaug[:, t, :, 0:DH],
                    acc.rearrange("p (h d) -> p h d", d=DH),
                )

            # ---- attention ---------------------------------------------------
            # outT feature-major [128 = head pair x 64, hp, N]
            outt_sb = sb.tile([128, 2, N], F32R)
            for hp in range(2):
                for qb in range(QB):
                    po = [
                        pso.tile([DH + 1, 512], F32, name="ps_o") for _ in range(2)
                    ]
                    for kc in range(NKC):
                        ps = psd.tile([128, 2, 512], F32, name="psd")
                        for hh in range(2):
                            base = hh * DH
                            nc.tensor.matmul(
                                ps[:, hh, :],
                                lhsT=kt_sb[
                                    base : base + DH, hp, kc * 128 : (kc + 1) * 128
                                ],
                                rhs=qt_sb[
                                    base : base + DH, hp, qb * 512 : (qb + 1) * 512
                                ],
                                start=True,
                                stop=True,
                            )
                        ex = expp.tile([128, 2, 512], F32R, name="expT")
                        nc.scalar.activation(
                            out=ex,
                            in_=ps,
                            func=mybir.ActivationFunctionType.Exp,
                            scale=SCALE,
                        )
                        for hh in range(2):
                            nc.tensor.matmul(
                                po[hh],
                                lhsT=vaug[:, kc, hp * 2 + hh, :],
                                rhs=ex[:, hh, :],
                                start=(kc == 0),
                                stop=(kc == NKC - 1),
                            )
                    for hh in range(2):
                        base = hh * DH
                        zinv = normp.tile([1, 512], F32R, name="zinv")
                        with nc.allow_low_precision(reason="f32r 1/Z"):
                            nc.vector.reciprocal(zinv, po[hh][DH : DH + 1, :])
                        ps_b = psbp.tile([DH, 512], F32, name="ps_b")
                        nc.tensor.matmul(
                            ps_b, lhsT=ones_r, rhs=zinv, start=True, stop=True
                        )
                        zb = normp.tile([DH, 512], F32, name="zb")
                        nc.vector.tensor_copy(zb, ps_b)
                        with nc.allow_low_precision(reason="f32r attn out"):
                            nc.vector.tensor_mul(
                                outt_sb[
                                    base : base + DH, hp, qb * 512 : (qb + 1) * 512
                                ],
                                po[hh][0:DH, :],
                                zb,
                            )

            # ---- AllGather heads within batch group -------------------------
            cc_in_r = cc_in.ap().rearrange("(m p) n -> p m n", p=128)
            for m in range(2):
                nc.sync.dma_start(out=cc_in_r[:, m, :], in_=outt_sb[:, m, :])
            nc.gpsimd.collective_compute(
                "AllGather",
                mybir.AluOpType.bypass,
                ins=[cc_in.ap().opt()],
                outs=[cc_out.ap().opt()],
                replica_groups=REPLICA_GROUPS,
            )
            ag_sb = big.tile([128, KC, N], F32R, tag="bigbuf")
            cc_out_r = cc_out.ap().rearrange("(c p) n -> p c n", p=128)
            for c in range(KC):
                nc.sync.dma_start(out=ag_sb[:, c, :], in_=cc_out_r[:, c, :])

            # ---- output projection ------------------------------------------
            for t in range(TT):
                ps = psd.tile([128, 2, 512], F32, name="psd")
                acc = ps[:, 0, 0:CS]
                for c in range(KC):
                    nc.tensor.matmul(
                        acc,
                        lhsT=ag_sb[:, c, t * 128 : (t + 1) * 128],
                        rhs=wo_sb[:, c, :],
                        start=(c == 0),
                        stop=(c == KC - 1),
                    )
                y_sb = yout.tile([128, CS], F32, name="y_sb")
                nc.vector.tensor_add(y_sb, acc, bias_sb)
                nc.sync.dma_start(out=y[t * 128 : (t + 1) * 128, :], in_=y_sb)

    nc.compile()
    return nc


_NC_CACHE = None


def _get_nc():
    global _NC_CACHE
    if _NC_CACHE is None:
        _NC_CACHE = build_nc()
    return _NC_CACHE


def _make_in_maps(x, w_qkv, w_out, b_out):
    in_maps = []
    for c in range(CORES):
        bi = c // GROUP_SIZE
        g = c % GROUP_SIZE
        cols = slice(g * CS, (g + 1) * CS)
        in_maps.append(
            {
                "xt": np.ascontiguousarray(x[bi].T),
                "wq": np.ascontiguousarray(w_qkv[:, cols]),
                "wk": np.ascontiguousarray(w_qkv[:, INNER:][:, cols]),
                "wv": np.ascontiguousarray(w_qkv[:, 2 * INNER:][:, cols]),
                "wo": np.ascontiguousarray(w_out[:, cols]),
                "bo": np.ascontiguousarray(b_out[cols]),
            }
        )
    return in_maps


def _assemble(results):
    out = np.empty((B, N, DIM), dtype=np.float32)
    for c in range(CORES):
        bi = c // GROUP_SIZE
        g = c % GROUP_SIZE
        out[bi, :, g * CS : (g + 1) * CS] = results[c]["y"]
    return out


def kernel(x, w_qkv, w_out, b_out, _trace=False, _trace_kwargs=None):
    x = np.asarray(x, dtype=np.float32)
    w_qkv = np.asarray(w_qkv, dtype=np.float32)
    w_out = np.asarray(w_out, dtype=np.float32)
    b_out = np.asarray(b_out, dtype=np.float32)
    nc = _get_nc()
    in_maps = _make_in_maps(x, w_qkv, w_out, b_out)
    res = run_bass_kernel_spmd(
        nc,
        in_maps,
        core_ids=list(range(CORES)),
        trace=_trace,
        **(_trace_kwargs or {}),
    )
    out = _assemble(res.results)
    if _trace:
        return out, res
    return out


# revision 7
# speedup vs baseline: 1.1388x; 1.1388x over previous
"""Multi-head attention (b=2, n=2048, dim=1024, 16 heads x 64) on 8 TRN2 NeuronCores.

Sharding: core c handles batch c//4 and heads 4*(c%4) .. 4*(c%4)+3
(data parallel over batch x 4-way head/tensor parallel). w_qkv is
column-sharded by head; w_out is column-sharded: each core computes a
256-column slice of the output after AllGathers of the attention outputs
within its 4-core batch group (no all-reduce needed).

Device layout is feature-major ("K-major"): x arrives pre-transposed
[dim, n] in bf16; Q^T/K^T are produced feature-major and V token-major
directly from the QKV projection (no on-chip transposes); attention
scores are computed transposed (dotsT[k, q]); softmax sums come from an
augmented ones-column in the V matmul; softmax exp runs on the scalar
engine with the 1/sqrt(d) scale folded in. The AllGather is split per
head-pair so the first gather overlaps the second half of attention, and
the output projection runs in two passes (one per gather). The final
output is produced transposed [cols, n]; the host transposes back.
"""

import sys

sys.path.insert(0, "/opt/trn_rl_repo")

import ml_dtypes
import numpy as np

import concourse.bass as bass
import concourse.tile as tile
from concourse import bacc, mybir
from concourse.bass_utils import run_bass_kernel_spmd

F32 = mybir.dt.float32
F32R = mybir.dt.float32r
BF16 = mybir.dt.bfloat16
NP_BF16 = np.dtype(ml_dtypes.bfloat16)

# Problem constants
B, N, DIM = 2, 2048, 1024
HEADS, DH = 16, 64
INNER = HEADS * DH
SCALE = DH ** -0.5
CORES = 8
GROUP_SIZE = 4
REPLICA_GROUPS = [[0, 1, 2, 3], [4, 5, 6, 7]]
HPC = 4  # heads per core
CS = HPC * DH  # 256 per-core feature columns

KC = DIM // 128  # 8 contraction chunks for dim
TT = N // 128  # 16 token tiles
QB = N // 512  # 4 q blocks
NKC = N // 128  # 16 key chunks


def build_nc():
    nc = bacc.Bacc("TRN2", target_bir_lowering=False, debug=False, num_devices=CORES)
    xt = nc.dram_tensor("xt", [DIM, N], BF16, kind="ExternalInput").ap()
    wq = nc.dram_tensor("wq", [DIM, CS], BF16, kind="ExternalInput").ap()
    wk = nc.dram_tensor("wk", [DIM, CS], BF16, kind="ExternalInput").ap()
    wv = nc.dram_tensor("wv", [DIM, CS], BF16, kind="ExternalInput").ap()
    wo = nc.dram_tensor("wo", [INNER, CS], BF16, kind="ExternalInput").ap()
    bo = nc.dram_tensor("bo", [CS], F32, kind="ExternalInput").ap()
    y = nc.dram_tensor("y", [CS, N], F32, kind="ExternalOutput").ap()  # y^T

    cc_in = [nc.dram_tensor(f"cc_in{m}", [128, N], BF16) for m in range(2)]
    cc_out = [nc.dram_tensor(f"cc_out{m}", [GROUP_SIZE * 128, N], BF16) for m in range(2)]

    with tile.TileContext(nc) as tc:
        with (
            tc.tile_pool(name="big", bufs=2) as big,  # xt, then the two AG results
            tc.tile_pool(name="sb", bufs=1) as sb,
            tc.tile_pool(name="expp", bufs=4) as expp,
            tc.tile_pool(name="yout", bufs=3) as yout,
            tc.tile_pool(name="norm", bufs=2) as normp,
            tc.tile_pool(name="psd", bufs=2, space="PSUM") as psd,
            tc.tile_pool(name="pso", bufs=2, space="PSUM") as pso,
            tc.tile_pool(name="psy", bufs=2, space="PSUM") as psyp,
        ):
            # ---- load inputs -------------------------------------------------
            xt_sb = big.tile([128, KC, N], BF16, tag="bigbuf")
            xt_r = xt.rearrange("(c p) n -> p c n", p=128)
            for c in range(KC):
                nc.sync.dma_start(out=xt_sb[:, c, :], in_=xt_r[:, c, :])
            wq_sb = sb.tile([128, KC, CS], BF16)
            wk_sb = sb.tile([128, KC, CS], BF16)
            wv_sb = sb.tile([128, KC, CS], BF16)
            wo_sb = sb.tile([128, KC, CS], BF16)
            nc.sync.dma_start(out=wq_sb, in_=wq.rearrange("(c p) n -> p c n", p=128))
            nc.sync.dma_start(out=wk_sb, in_=wk.rearrange("(c p) n -> p c n", p=128))
            nc.sync.dma_start(out=wv_sb, in_=wv.rearrange("(c p) n -> p c n", p=128))
            nc.sync.dma_start(out=wo_sb, in_=wo.rearrange("(c p) n -> p c n", p=128))

            # bias, transposed layout: partition = column-within-block
            bias_sb = sb.tile([128, 2], F32)
            nc.sync.dma_start(out=bias_sb, in_=bo.rearrange("(cb p) -> p cb", p=128))

            ones_f = sb.tile([128, TT], F32)
            nc.vector.memset(ones_f, 1.0)
            ones_r = sb.tile([1, DH], F32R)
            nc.vector.tensor_copy(ones_r, ones_f[0:1, 0:1].broadcast_to([1, DH]))

            # ---- QKV projection ---------------------------------------------
            qt_sb = sb.tile([128, 2, N], BF16)
            kt_sb = sb.tile([128, 2, N], BF16)
            vaug = sb.tile([128, TT, HPC, DH + 1], BF16)
            with nc.allow_low_precision(reason="bf16 ones column"):
                for h in range(HPC):
                    nc.vector.tensor_copy(vaug[:, :, h, DH], ones_f)

            for m in range(2):  # head pairs
                for qb in range(QB):
                    for dst, w_sb in ((qt_sb, wq_sb), (kt_sb, wk_sb)):
                        ps = psd.tile([128, 2, 512], F32, name="psd")
                        acc = ps[:, 0, :]
                        for c in range(KC):
                            nc.tensor.matmul(
                                acc,
                                lhsT=w_sb[:, c, m * 128 : (m + 1) * 128],
                                rhs=xt_sb[:, c, qb * 512 : (qb + 1) * 512],
                                start=(c == 0),
                                stop=(c == KC - 1),
                            )
                        with nc.allow_low_precision(reason="bf16 attention"):
                            nc.vector.tensor_copy(
                                dst[:, m, qb * 512 : (qb + 1) * 512], acc
                            )
            for t in range(TT):
                ps = psd.tile([128, 2, 512], F32, name="psd")
                acc = ps[:, 0, 0:CS]
                for c in range(KC):
                    nc.tensor.matmul(
                        acc,
                        lhsT=xt_sb[:, c, t * 128 : (t + 1) * 128],
                        rhs=wv_sb[:, c, :],
                        start=(c == 0),
                        stop=(c == KC - 1),
                    )
                with nc.allow_low_precision(reason="bf16 attention"):
                    nc.vector.tensor_copy(
                        vaug[:, t, :, 0:DH],
                        acc.rearrange("p (h d) -> p h d", d=DH),
                    )

            # ---- attention (+ per-head-pair AllGather) ----------------------
            outt_sb = sb.tile([128, 2, N], BF16)

            def attend(hp):
                for qb in range(QB):
                    po = [pso.tile([DH + 1, 512], F32, name="ps_o") for _ in range(2)]
                    for kc in range(NKC):
                        ps = psd.tile([128, 2, 512], F32, name="psd")
                        for hh in range(2):
                            base = hh * DH
                            nc.tensor.matmul(
                                ps[:, hh, :],
                                lhsT=kt_sb[
                                    base : base + DH, hp, kc * 128 : (kc + 1) * 128
                                ],
                                rhs=qt_sb[
                                    base : base + DH, hp, qb * 512 : (qb + 1) * 512
                                ],
                                start=True,
                                stop=True,
                            )
                        ex = expp.tile([128, 2, 512], BF16, name="expT")
                        nc.scalar.activation(
                            out=ex,
                            in_=ps,
                            func=mybir.ActivationFunctionType.Exp,
                            scale=SCALE,
                        )
                        for hh in range(2):
                            nc.tensor.matmul(
                                po[hh],
                                lhsT=vaug[:, kc, hp * 2 + hh, :],
                                rhs=ex[:, hh, :],
                                start=(kc == 0),
                                stop=(kc == NKC - 1),
                            )
                    for hh in range(2):
                        base = hh * DH
                        po_sb = normp.tile([DH + 1, 512], F32, name="po_sb")
                        nc.vector.tensor_copy(po_sb, po[hh])
                        zinv = normp.tile([1, 512], F32R, name="zinv")
                        with nc.allow_low_precision(reason="f32r 1/Z"):
                            nc.vector.reciprocal(zinv, po_sb[DH : DH + 1, :])
                        ps_b = psyp.tile([128, 512], F32, name="psy")
                        nc.tensor.matmul(
                            ps_b[0:DH, :], lhsT=ones_r, rhs=zinv, start=True, stop=True
                        )
                        with nc.allow_low_precision(reason="bf16 attention out"):
                            nc.vector.tensor_mul(
                                outt_sb[
                                    base : base + DH, hp, qb * 512 : (qb + 1) * 512
                                ],
                                po_sb[0:DH, :],
                                ps_b[0:DH, :],
                            )
                # AllGather this head pair within the batch group
                nc.sync.dma_start(out=cc_in[hp].ap(), in_=outt_sb[:, hp, :])
                nc.gpsimd.collective_compute(
                    "AllGather",
                    mybir.AluOpType.bypass,
                    ins=[cc_in[hp].ap().opt()],
                    outs=[cc_out[hp].ap().opt()],
                    replica_groups=REPLICA_GROUPS,
                )

            attend(0)
            attend(1)


            # ---- output projection (two passes, one per AllGather) ----------
            # pass A: contributions of AG0 (heads {4r, 4r+1}); host permutes
            # w_out rows to match [AG0 r0..r3, AG1 r0..r3] chunk order.
            y_acc = sb.tile([128, 2, N], F32)
            ag0 = big.tile([128, 4, N], BF16, tag="bigbuf")
            cc0_r = cc_out[0].ap().rearrange("(c p) n -> p c n", p=128)
            for c in range(4):
                nc.sync.dma_start(out=ag0[:, c, :], in_=cc0_r[:, c, :])
            for cb in range(2):
                for qb in range(QB):
                    ps = psyp.tile([128, 512], F32, name="psy")
                    for c in range(4):
                        nc.tensor.matmul(
                            ps,
                            lhsT=wo_sb[:, c, cb * 128 : (cb + 1) * 128],
                            rhs=ag0[:, c, qb * 512 : (qb + 1) * 512],
                            start=(c == 0),
                            stop=(c == 3),
                        )
                    nc.vector.tensor_copy(y_acc[:, cb, qb * 512 : (qb + 1) * 512], ps)

            ag1 = big.tile([128, 4, N], BF16, tag="bigbuf")
            cc1_r = cc_out[1].ap().rearrange("(c p) n -> p c n", p=128)
            for c in range(4):
                nc.sync.dma_start(out=ag1[:, c, :], in_=cc1_r[:, c, :])
            for cb in range(2):
                for qb in range(QB):
                    ps = psyp.tile([128, 512], F32, name="psy")
                    for c in range(4):
                        nc.tensor.matmul(
                            ps,
                            lhsT=wo_sb[:, 4 + c, cb * 128 : (cb + 1) * 128],
                            rhs=ag1[:, c, qb * 512 : (qb + 1) * 512],
                            start=(c == 0),
                            stop=(c == 3),
                        )
                    y_sb = yout.tile([128, 512], F32, name="y_sb")
                    nc.vector.tensor_add(
                        y_sb, ps, y_acc[:, cb, qb * 512 : (qb + 1) * 512]
                    )
                    nc.vector.tensor_scalar_add(
                        out=y_sb, in0=y_sb, scalar1=bias_sb[:, cb : cb + 1]
                    )
                    nc.sync.dma_start(
                        out=y[cb * 128 : (cb + 1) * 128, qb * 512 : (qb + 1) * 512],
                        in_=y_sb,
                    )

    nc.compile()
    return nc


_NC_CACHE = None


def _get_nc():
    global _NC_CACHE
    if _NC_CACHE is None:
        _NC_CACHE = build_nc()
    return _NC_CACHE


def _wo_perm(w_out):
    # chunk order [AG0: r0..r3 -> w_out rows 256r..256r+128,
    #              AG1: r0..r3 -> w_out rows 256r+128..256r+256]
    blocks = [w_out[256 * r : 256 * r + 128] for r in range(4)]
    blocks += [w_out[256 * r + 128 : 256 * r + 256] for r in range(4)]
    return np.concatenate(blocks, axis=0)


def _make_in_maps(x, w_qkv, w_out, b_out):
    wop = _wo_perm(w_out)
    in_maps = []
    for c in range(CORES):
        bi = c // GROUP_SIZE
        g = c % GROUP_SIZE
        cols = slice(g * CS, (g + 1) * CS)
        in_maps.append(
            {
                "xt": np.ascontiguousarray(x[bi].T).astype(NP_BF16),
                "wq": np.ascontiguousarray(w_qkv[:, cols]).astype(NP_BF16),
                "wk": np.ascontiguousarray(w_qkv[:, INNER:][:, cols]).astype(NP_BF16),
                "wv": np.ascontiguousarray(w_qkv[:, 2 * INNER:][:, cols]).astype(
                    NP_BF16
                ),
                "wo": np.ascontiguousarray(wop[:, cols]).astype(NP_BF16),
                "bo": np.ascontiguousarray(b_out[cols]),
            }
        )
    return in_maps


def _assemble(results):
    out = np.empty((B, N, DIM), dtype=np.float32)
    for c in range(CORES):
        bi = c // GROUP_SIZE
        g = c % GROUP_SIZE
        out[bi, :, g * CS : (g + 1) * CS] = results[c]["y"].T
    return out


def kernel(x, w_qkv, w_out, b_out, _trace=False, _trace_kwargs=None):
    x = np.asarray(x, dtype=np.float32)
    w_qkv = np.asarray(w_qkv, dtype=np.float32)
    w_out = np.asarray(w_out, dtype=np.float32)
    b_out = np.asarray(b_out, dtype=np.float32)
    nc = _get_nc()
    in_maps = _make_in_maps(x, w_qkv, w_out, b_out)
    res = run_bass_kernel_spmd(
        nc,
        in_maps,
        core_ids=list(range(CORES)),
        trace=_trace,
        **(_trace_kwargs or {}),
    )
    out = _assemble(res.results)
    if _trace:
        return out, res
    return out


# revision 10
# speedup vs baseline: 1.2713x; 1.1163x over previous
"""Multi-head attention (b=2, n=2048, dim=1024, 16 heads x 64) on 8 TRN2 NeuronCores.

Sharding: core c handles batch c//4 and heads 4*(c%4) .. 4*(c%4)+3
(data parallel over batch x 4-way head/tensor parallel). w_qkv is
column-sharded by head; w_out is column-sharded: each core computes a
256-column slice of the output after AllGathers of the attention outputs
within its 4-core batch group (no all-reduce needed).

Device layout is feature-major ("K-major"): x arrives pre-transposed
[dim, n] in bf16; Q^T/K^T are produced feature-major and V token-major
directly from the QKV projection (no on-chip transposes); attention
scores are computed transposed (dotsT[k, q]); softmax sums come from an
augmented ones-column in the V matmul; softmax exp runs on the scalar
engine with the 1/sqrt(d) scale folded in. The AllGather is split per
head-pair so the first gather overlaps the second half of attention, and
the output projection runs in two passes (one per gather). The final
output is produced transposed [cols, n]; the host transposes back.
"""

import sys

sys.path.insert(0, "/opt/trn_rl_repo")

import ml_dtypes
import numpy as np

import concourse.bass as bass
import concourse.tile as tile
from concourse import bacc, mybir
from concourse.bass_utils import run_bass_kernel_spmd

F32 = mybir.dt.float32
F32R = mybir.dt.float32r
BF16 = mybir.dt.bfloat16
NP_BF16 = np.dtype(ml_dtypes.bfloat16)

# Problem constants
B, N, DIM = 2, 2048, 1024
HEADS, DH = 16, 64
INNER = HEADS * DH
SCALE = DH ** -0.5
CORES = 8
GROUP_SIZE = 4
REPLICA_GROUPS = [[0, 1, 2, 3], [4, 5, 6, 7]]
HPC = 4  # heads per core
CS = HPC * DH  # 256 per-core feature columns

KC = DIM // 128  # 8 contraction chunks for dim
TT = N // 128  # 16 token tiles
QB = N // 512  # 4 q blocks
NKC = N // 128  # 16 key chunks


def build_nc():
    nc = bacc.Bacc("TRN2", target_bir_lowering=False, debug=False, num_devices=CORES)
    xt = nc.dram_tensor("xt", [DIM, N], BF16, kind="ExternalInput").ap()
    wq = nc.dram_tensor("wq", [DIM, CS], BF16, kind="ExternalInput").ap()
    wk = nc.dram_tensor("wk", [DIM, CS], BF16, kind="ExternalInput").ap()
    wv = nc.dram_tensor("wv", [DIM, CS], BF16, kind="ExternalInput").ap()
    wo = nc.dram_tensor("wo", [INNER, CS], BF16, kind="ExternalInput").ap()
    bo = nc.dram_tensor("bo", [CS], F32, kind="ExternalInput").ap()
    y = nc.dram_tensor("y", [CS, N], F32, kind="ExternalOutput").ap()  # y^T

    cc_in = [[nc.dram_tensor(f"cc_in{m}_{q}", [128, N // 2], BF16) for q in range(2)] for m in range(2)]
    cc_out = [[nc.dram_tensor(f"cc_out{m}_{q}", [GROUP_SIZE * 128, N // 2], BF16) for q in range(2)] for m in range(2)]

    with tile.TileContext(nc) as tc:
        with (
            tc.tile_pool(name="big", bufs=2) as big,  # xt, then the two AG results
            tc.tile_pool(name="sb", bufs=1) as sb,
            tc.tile_pool(name="expp", bufs=4) as expp,
            tc.tile_pool(name="yout", bufs=3) as yout,
            tc.tile_pool(name="norm", bufs=2) as normp,
            tc.tile_pool(name="psd", bufs=2, space="PSUM") as psd,
            tc.tile_pool(name="pso", bufs=2, space="PSUM") as pso,
            tc.tile_pool(name="psy", bufs=2, space="PSUM") as psyp,
        ):
            # ---- load inputs -------------------------------------------------
            xt_sb = big.tile([128, KC, N], BF16, tag="bigbuf")
            xt_r = xt.rearrange("(c p) n -> p c n", p=128)
            for c in range(KC):
                nc.sync.dma_start(out=xt_sb[:, c, :], in_=xt_r[:, c, :])
            wq_sb = sb.tile([128, KC, CS], BF16)
            wk_sb = sb.tile([128, KC, CS], BF16)
            wv_sb = sb.tile([128, KC, CS], BF16)
            wo_sb = sb.tile([128, KC, CS], BF16)
            nc.sync.dma_start(out=wq_sb, in_=wq.rearrange("(c p) n -> p c n", p=128))
            nc.sync.dma_start(out=wk_sb, in_=wk.rearrange("(c p) n -> p c n", p=128))
            nc.sync.dma_start(out=wv_sb, in_=wv.rearrange("(c p) n -> p c n", p=128))
            nc.sync.dma_start(out=wo_sb, in_=wo.rearrange("(c p) n -> p c n", p=128))

            # bias, transposed layout: partition = column-within-block
            bias_sb = sb.tile([128, 2], F32)
            nc.sync.dma_start(out=bias_sb, in_=bo.rearrange("(cb p) -> p cb", p=128))

            ones_f = sb.tile([128, TT], F32)
            nc.vector.memset(ones_f, 1.0)
            ones_r = sb.tile([1, DH], F32R)
            nc.vector.tensor_copy(ones_r, ones_f[0:1, 0:1].broadcast_to([1, DH]))

            # ---- QKV projection ---------------------------------------------
            qt_sb = sb.tile([128, 2, N], BF16)
            kt_sb = sb.tile([128, 2, N], BF16)
            vaug = sb.tile([128, TT, HPC, DH + 1], BF16)
            with nc.allow_low_precision(reason="bf16 ones column"):
                for h in range(HPC):
                    nc.vector.tensor_copy(vaug[:, :, h, DH], ones_f)

            for m in range(2):  # head pairs
                for qb in range(QB):
                    for dst, w_sb in ((qt_sb, wq_sb), (kt_sb, wk_sb)):
                        ps = psd.tile([128, 2, 512], F32, name="psd")
                        acc = ps[:, 0, :]
                        for c in range(KC):
                            nc.tensor.matmul(
                                acc,
                                lhsT=w_sb[:, c, m * 128 : (m + 1) * 128],
                                rhs=xt_sb[:, c, qb * 512 : (qb + 1) * 512],
                                start=(c == 0),
                                stop=(c == KC - 1),
                            )
                        with nc.allow_low_precision(reason="bf16 attention"):
                            nc.vector.tensor_copy(
                                dst[:, m, qb * 512 : (qb + 1) * 512], acc
                            )
            for t in range(TT):
                ps = psd.tile([128, 2, 512], F32, name="psd")
                acc = ps[:, 0, 0:CS]
                for c in range(KC):
                    nc.tensor.matmul(
                        acc,
                        lhsT=xt_sb[:, c, t * 128 : (t + 1) * 128],
                        rhs=wv_sb[:, c, :],
                        start=(c == 0),
                        stop=(c == KC - 1),
                    )
                with nc.allow_low_precision(reason="bf16 attention"):
                    nc.vector.tensor_copy(
                        vaug[:, t, :, 0:DH],
                        acc.rearrange("p (h d) -> p h d", d=DH),
                    )

            # ---- attention (+ per-head-pair AllGather) ----------------------
            outt_sb = sb.tile([128, 2, N], BF16)

            def attend(hp):
                for qb in range(QB):
                    po = [pso.tile([DH + 1, 512], F32, name="ps_o") for _ in range(2)]
                    for kc in range(NKC):
                        ps = psd.tile([128, 2, 512], F32, name="psd")
                        for hh in range(2):
                            base = hh * DH
                            nc.tensor.matmul(
                                ps[:, hh, :],
                                lhsT=kt_sb[
                                    base : base + DH, hp, kc * 128 : (kc + 1) * 128
                                ],
                                rhs=qt_sb[
                                    base : base + DH, hp, qb * 512 : (qb + 1) * 512
                                ],
                                start=True,
                                stop=True,
                            )
                        ex = expp.tile([128, 2, 512], BF16, name="expT")
                        nc.scalar.activation(
                            out=ex,
                            in_=ps,
                            func=mybir.ActivationFunctionType.Exp,
                            scale=SCALE,
                        )
                        for hh in range(2):
                            nc.tensor.matmul(
                                po[hh],
                                lhsT=vaug[:, kc, hp * 2 + hh, :],
                                rhs=ex[:, hh, :],
                                start=(kc == 0),
                                stop=(kc == NKC - 1),
                            )
                    for hh in range(2):
                        base = hh * DH
                        po_sb = normp.tile([DH + 1, 512], F32, name="po_sb")
                        nc.vector.tensor_copy(po_sb, po[hh])
                        zinv2 = normp.tile([1, 512], F32R, name="zinv2")
                        with nc.allow_low_precision(reason="f32r 1/Z"):
                            nc.vector.reciprocal(zinv2, po_sb[DH : DH + 1, :])
                        ps_b = psyp.tile([128, 512], F32, name="psy")
                        nc.tensor.matmul(
                            ps_b[0:DH, :], lhsT=ones_r, rhs=zinv2, start=True, stop=True
                        )
                        with nc.allow_low_precision(reason="bf16 attention out"):
                            nc.vector.tensor_mul(
                                outt_sb[
                                    base : base + DH, hp, qb * 512 : (qb + 1) * 512
                                ],
                                po_sb[0:DH, :],
                                ps_b[0:DH, :],
                            )
                    if qb % 2 == 1:
                        # AllGather this (head pair, token half) within the group
                        qh = qb // 2
                        sl = slice(qh * 1024, (qh + 1) * 1024)
                        nc.gpsimd.dma_start(
                            out=cc_in[hp][qh].ap(), in_=outt_sb[:, hp, sl]
                        )
                        nc.gpsimd.collective_compute(
                            "AllGather",
                            mybir.AluOpType.bypass,
                            ins=[cc_in[hp][qh].ap().opt()],
                            outs=[cc_out[hp][qh].ap().opt()],
                            replica_groups=REPLICA_GROUPS,
                        )
                        nc.sync.dma_start(
                            out=ag_all[:, hp, qh, :, :],
                            in_=cc_out[hp][qh].ap().rearrange(
                                "(c p) n -> p c n", p=128
                            ),
                        )

            ag_all = big.tile([128, 2, 2, GROUP_SIZE, N // 2], BF16, tag="bigbuf")
            attend(0)
            attend(1)


            # ---- output projection (4 passes: token half x head-pair AG) ----
            # host permutes w_out rows to [hp0: r0..r3, hp1: r0..r3] chunks.
            y_acc = sb.tile([128, 2, N], F32)
            for qh in range(2):
                for hp in range(2):
                    for cb in range(2):
                        for q2 in range(2):
                            qb = qh * 2 + q2
                            ps = psyp.tile([128, 512], F32, name="psy")
                            for c in range(4):
                                nc.tensor.matmul(
                                    ps,
                                    lhsT=wo_sb[
                                        :, hp * 4 + c, cb * 128 : (cb + 1) * 128
                                    ],
                                    rhs=ag_all[
                                        :, hp, qh, c, q2 * 512 : (q2 + 1) * 512
                                    ],
                                    start=(c == 0),
                                    stop=(c == 3),
                                )
                            if hp == 0:
                                nc.vector.tensor_copy(
                                    y_acc[:, cb, qb * 512 : (qb + 1) * 512], ps
                                )
                            else:
                                y_sb = yout.tile([128, 512], F32, name="y_sb")
                                nc.vector.tensor_add(
                                    y_sb, ps, y_acc[:, cb, qb * 512 : (qb + 1) * 512]
                                )
                                nc.vector.tensor_scalar_add(
                                    out=y_sb, in0=y_sb, scalar1=bias_sb[:, cb : cb + 1]
                                )
                                nc.sync.dma_start(
                                    out=y[
                                        cb * 128 : (cb + 1) * 128,
                                        qb * 512 : (qb + 1) * 512,
                                    ],
                                    in_=y_sb,
                                )

    nc.compile()
    return nc


_NC_CACHE = None


def _get_nc():
    global _NC_CACHE
    if _NC_CACHE is None:
        _NC_CACHE = build_nc()
    return _NC_CACHE


def _wo_perm(w_out):
    # chunk order [AG0: r0..r3 -> w_out rows 256r..256r+128,
    #              AG1: r0..r3 -> w_out rows 256r+128..256r+256]
    blocks = [w_out[256 * r : 256 * r + 128] for r in range(4)]
    blocks += [w_out[256 * r + 128 : 256 * r + 256] for r in range(4)]
    return np.concatenate(blocks, axis=0)


def _make_in_maps(x, w_qkv, w_out, b_out):
    wop = _wo_perm(w_out)
    in_maps = []
    for c in range(CORES):
        bi = c // GROUP_SIZE
        g = c % GROUP_SIZE
        cols = slice(g * CS, (g + 1) * CS)
        in_maps.append(
            {
                "xt": np.ascontiguousarray(x[bi].T).astype(NP_BF16),
                "wq": np.ascontiguousarray(w_qkv[:, cols]).astype(NP_BF16),
                "wk": np.ascontiguousarray(w_qkv[:, INNER:][:, cols]).astype(NP_BF16),
                "wv": np.ascontiguousarray(w_qkv[:, 2 * INNER:][:, cols]).astype(
                    NP_BF16
                ),
                "wo": np.ascontiguousarray(wop[:, cols]).astype(NP_BF16),
                "bo": np.ascontiguousarray(b_out[cols]),
            }
        )
    return in_maps


def _assemble(results):
    out = np.empty((B, N, DIM), dtype=np.float32)
    for c in range(CORES):
        bi = c // GROUP_SIZE
        g = c % GROUP_SIZE
        out[bi, :, g * CS : (g + 1) * CS] = results[c]["y"].T
    return out


def kernel(x, w_qkv, w_out, b_out, _trace=False, _trace_kwargs=None):
    x = np.asarray(x, dtype=np.float32)
    w_qkv = np.asarray(w_qkv, dtype=np.float32)
    w_out = np.asarray(w_out, dtype=np.float32)
    b_out = np.asarray(b_out, dtype=np.float32)
    nc = _get_nc()
    in_maps = _make_in_maps(x, w_qkv, w_out, b_out)
    res = run_bass_kernel_spmd(
        nc,
        in_maps,
        core_ids=list(range(CORES)),
        trace=_trace,
        **(_trace_kwargs or {}),
    )
    out = _assemble(res.results)
    if _trace:
        return out, res
    return out


# revision 11
# speedup vs baseline: 1.5828x; 1.2451x over previous
"""Multi-head attention (b=2, n=2048, dim=1024, 16 heads x 64) on 8 TRN2 NeuronCores.

Sharding: core c handles batch c//4 and heads 4*(c%4) .. 4*(c%4)+3
(data parallel over batch x 4-way head/tensor parallel). w_qkv is
column-sharded by head; w_out is column-sharded: each core computes a
256-column slice of the output after AllGathers of the attention outputs
within its 4-core batch group (no all-reduce needed).

Device layout is feature-major ("K-major"): x arrives pre-transposed
[dim, n] in bf16; Q^T/K^T are produced feature-major and V token-major
directly from the QKV projection (no on-chip transposes); attention
scores are computed transposed (dotsT[k, q]); softmax sums come from an
augmented ones-column in the V matmul; softmax exp runs on the scalar
engine with the 1/sqrt(d) scale folded in. The AllGather is split per
head-pair so the first gather overlaps the second half of attention, and
the output projection runs in two passes (one per gather). The final
output is produced transposed [cols, n]; the host transposes back.
"""

import sys

sys.path.insert(0, "/opt/trn_rl_repo")

import ml_dtypes
import numpy as np

import concourse.bass as bass
import concourse.tile as tile
from concourse import bacc, mybir
from concourse.bass_utils import run_bass_kernel_spmd

F32 = mybir.dt.float32
F32R = mybir.dt.float32r
BF16 = mybir.dt.bfloat16
NP_BF16 = np.dtype(ml_dtypes.bfloat16)

# Problem constants
B, N, DIM = 2, 2048, 1024
HEADS, DH = 16, 64
INNER = HEADS * DH
SCALE = DH ** -0.5
CORES = 8
GROUP_SIZE = 4
REPLICA_GROUPS = [[0, 1, 2, 3], [4, 5, 6, 7]]
HPC = 4  # heads per core
CS = HPC * DH  # 256 per-core feature columns

KC = DIM // 128  # 8 contraction chunks for dim
TT = N // 128  # 16 token tiles
QB = N // 512  # 4 q blocks
NKC = N // 128  # 16 key chunks


def build_nc():
    nc = bacc.Bacc("TRN2", target_bir_lowering=False, debug=False, num_devices=CORES)
    xt = nc.dram_tensor("xt", [DIM, N], BF16, kind="ExternalInput").ap()
    wq = nc.dram_tensor("wq", [DIM, CS], BF16, kind="ExternalInput").ap()
    wk = nc.dram_tensor("wk", [DIM, CS], BF16, kind="ExternalInput").ap()
    wv = nc.dram_tensor("wv", [DIM, CS], BF16, kind="ExternalInput").ap()
    wo = nc.dram_tensor("wo", [INNER, CS], BF16, kind="ExternalInput").ap()
    bo = nc.dram_tensor("bo", [CS], F32, kind="ExternalInput").ap()
    y = nc.dram_tensor("y", [CS, N], F32, kind="ExternalOutput").ap()  # y^T

    cc_in = [[nc.dram_tensor(f"cc_in{m}_{q}", [128, N // 2], BF16) for q in range(2)] for m in range(2)]
    cc_out = [[nc.dram_tensor(f"cc_out{m}_{q}", [GROUP_SIZE * 128, N // 2], BF16) for q in range(2)] for m in range(2)]

    with tile.TileContext(nc) as tc:
        with (
            tc.tile_pool(name="big", bufs=2) as big,  # xt, then the two AG results
            tc.tile_pool(name="sb", bufs=1) as sb,
            tc.tile_pool(name="expp", bufs=4) as expp,
            tc.tile_pool(name="yout", bufs=3) as yout,
            tc.tile_pool(name="norm", bufs=2) as normp,
            tc.tile_pool(name="psd", bufs=2, space="PSUM") as psd,
            tc.tile_pool(name="pso", bufs=2, space="PSUM") as pso,
            tc.tile_pool(name="psy", bufs=2, space="PSUM") as psyp,
        ):
            # ---- load inputs -------------------------------------------------
            xt_sb = big.tile([128, KC, N], BF16, tag="bigbuf")
            xt_r = xt.rearrange("(c p) n -> p c n", p=128)
            for c in range(KC):
                nc.sync.dma_start(out=xt_sb[:, c, :], in_=xt_r[:, c, :])
            wq_sb = sb.tile([128, KC, CS], BF16)
            wk_sb = sb.tile([128, KC, CS], BF16)
            wv_sb = sb.tile([128, KC, CS], BF16)
            wo_sb = sb.tile([128, KC, CS], BF16)
            nc.sync.dma_start(out=wq_sb, in_=wq.rearrange("(c p) n -> p c n", p=128))
            nc.sync.dma_start(out=wk_sb, in_=wk.rearrange("(c p) n -> p c n", p=128))
            nc.sync.dma_start(out=wv_sb, in_=wv.rearrange("(c p) n -> p c n", p=128))
            nc.sync.dma_start(out=wo_sb, in_=wo.rearrange("(c p) n -> p c n", p=128))

            # bias, transposed layout: partition = column-within-block
            bias_sb = sb.tile([128, 2], F32)
            nc.sync.dma_start(out=bias_sb, in_=bo.rearrange("(cb p) -> p cb", p=128))

            ones_f = sb.tile([128, TT], F32)
            nc.vector.memset(ones_f, 1.0)
            ones_r = sb.tile([1, DH], F32R)
            nc.vector.tensor_copy(ones_r, ones_f[0:1, 0:1].broadcast_to([1, DH]))

            # ---- QKV projection ---------------------------------------------
            qt_sb = sb.tile([128, 2, N], BF16)
            kt_sb = sb.tile([128, 2, N], BF16)
            vaug = sb.tile([128, TT, HPC, DH + 1], BF16)
            with nc.allow_low_precision(reason="bf16 ones column"):
                for h in range(HPC):
                    nc.vector.tensor_copy(vaug[:, :, h, DH], ones_f)

            for m in range(2):  # head pairs
                for qb in range(QB):
                    for dst, w_sb in ((qt_sb, wq_sb), (kt_sb, wk_sb)):
                        ps = psd.tile([128, 2, 512], F32, name="psd")
                        acc = ps[:, 0, :]
                        for c in range(KC):
                            nc.tensor.matmul(
                                acc,
                                lhsT=w_sb[:, c, m * 128 : (m + 1) * 128],
                                rhs=xt_sb[:, c, qb * 512 : (qb + 1) * 512],
                                start=(c == 0),
                                stop=(c == KC - 1),
                            )
                        with nc.allow_low_precision(reason="bf16 attention"):
                            nc.vector.tensor_copy(
                                dst[:, m, qb * 512 : (qb + 1) * 512], acc
                            )
            for t in range(TT):
                ps = psd.tile([128, 2, 512], F32, name="psd")
                acc = ps[:, 0, 0:CS]
                for c in range(KC):
                    nc.tensor.matmul(
                        acc,
                        lhsT=xt_sb[:, c, t * 128 : (t + 1) * 128],
                        rhs=wv_sb[:, c, :],
                        start=(c == 0),
                        stop=(c == KC - 1),
                    )
                with nc.allow_low_precision(reason="bf16 attention"):
                    nc.vector.tensor_copy(
                        vaug[:, t, :, 0:DH],
                        acc.rearrange("p (h d) -> p h d", d=DH),
                    )

            # ---- attention (+ per-head-pair AllGather) ----------------------
            outt_sb = sb.tile([128, 2, N], BF16)

            def attend(hp):
                for qb in range(QB):
                    po = [pso.tile([DH + 1, 512], F32, name="ps_o") for _ in range(2)]
                    # software-pipelined: attV for chunk kc-1 issues after the
                    # dots matmuls for chunk kc, so the in-order PE stream
                    # never stalls on the exp of the current chunk.
                    pend = None
                    for kc in range(NKC):
                        ps = psd.tile([128, 2, 512], F32, name="psd")
                        for hh in range(2):
                            base = hh * DH
                            nc.tensor.matmul(
                                ps[:, hh, :],
                                lhsT=kt_sb[
                                    base : base + DH, hp, kc * 128 : (kc + 1) * 128
                                ],
                                rhs=qt_sb[
                                    base : base + DH, hp, qb * 512 : (qb + 1) * 512
                                ],
                                start=True,
                                stop=True,
                            )
                        if pend is not None:
                            pkc, pex = pend
                            for hh in range(2):
                                nc.tensor.matmul(
                                    po[hh],
                                    lhsT=vaug[:, pkc, hp * 2 + hh, :],
                                    rhs=pex[:, hh, :],
                                    start=(pkc == 0),
                                    stop=False,
                                )
                        ex = expp.tile([128, 2, 512], BF16, name="expT")
                        nc.scalar.activation(
                            out=ex,
                            in_=ps,
                            func=mybir.ActivationFunctionType.Exp,
                            scale=SCALE,
                        )
                        pend = (kc, ex)
                    pkc, pex = pend
                    for hh in range(2):
                        nc.tensor.matmul(
                            po[hh],
                            lhsT=vaug[:, pkc, hp * 2 + hh, :],
                            rhs=pex[:, hh, :],
                            start=False,
                            stop=True,
                        )
                    for hh in range(2):
                        base = hh * DH
                        po_sb = normp.tile([DH + 1, 512], F32, name="po_sb")
                        nc.vector.tensor_copy(po_sb, po[hh])
                        zinv2 = normp.tile([1, 512], F32R, name="zinv2")
                        with nc.allow_low_precision(reason="f32r 1/Z"):
                            nc.vector.reciprocal(zinv2, po_sb[DH : DH + 1, :])
                        ps_b = psyp.tile([128, 512], F32, name="psy")
                        nc.tensor.matmul(
                            ps_b[0:DH, :], lhsT=ones_r, rhs=zinv2, start=True, stop=True
                        )
                        with nc.allow_low_precision(reason="bf16 attention out"):
                            nc.vector.tensor_mul(
                                outt_sb[
                                    base : base + DH, hp, qb * 512 : (qb + 1) * 512
                                ],
                                po_sb[0:DH, :],
                                ps_b[0:DH, :],
                            )
                    if qb % 2 == 1:
                        # AllGather this (head pair, token half) within the group
                        qh = qb // 2
                        sl = slice(qh * 1024, (qh + 1) * 1024)
                        nc.gpsimd.dma_start(
                            out=cc_in[hp][qh].ap(), in_=outt_sb[:, hp, sl]
                        )
                        nc.gpsimd.collective_compute(
                            "AllGather",
                            mybir.AluOpType.bypass,
                            ins=[cc_in[hp][qh].ap().opt()],
                            outs=[cc_out[hp][qh].ap().opt()],
                            replica_groups=REPLICA_GROUPS,
                        )
                        nc.sync.dma_start(
                            out=ag_all[:, hp, qh, :, :],
                            in_=cc_out[hp][qh].ap().rearrange(
                                "(c p) n -> p c n", p=128
                            ),
                        )

            ag_all = big.tile([128, 2, 2, GROUP_SIZE, N // 2], BF16, tag="bigbuf")
            attend(0)
            attend(1)


            # ---- output projection (4 passes: token half x head-pair AG) ----
            # host permutes w_out rows to [hp0: r0..r3, hp1: r0..r3] chunks.
            y_acc = sb.tile([128, 2, N], F32)
            for qh in range(2):
                for hp in range(2):
                    for cb in range(2):
                        for q2 in range(2):
                            qb = qh * 2 + q2
                            ps = psyp.tile([128, 512], F32, name="psy")
                            for c in range(4):
                                nc.tensor.matmul(
                                    ps,
                                    lhsT=wo_sb[
                                        :, hp * 4 + c, cb * 128 : (cb + 1) * 128
                                    ],
                                    rhs=ag_all[
                                        :, hp, qh, c, q2 * 512 : (q2 + 1) * 512
                                    ],
                                    start=(c == 0),
                                    stop=(c == 3),
                                )
                            if hp == 0:
                                nc.vector.tensor_copy(
                                    y_acc[:, cb, qb * 512 : (qb + 1) * 512], ps
                                )
                            else:
                                y_sb = yout.tile([128, 512], F32, name="y_sb")
                                nc.vector.tensor_add(
                                    y_sb, ps, y_acc[:, cb, qb * 512 : (qb + 1) * 512]
                                )
                                nc.vector.tensor_scalar_add(
                                    out=y_sb, in0=y_sb, scalar1=bias_sb[:, cb : cb + 1]
                                )
                                nc.sync.dma_start(
                                    out=y[
                                        cb * 128 : (cb + 1) * 128,
                                        qb * 512 : (qb + 1) * 512,
                                    ],
                                    in_=y_sb,
                                )

    nc.compile()
    return nc


_NC_CACHE = None


def _get_nc():
    global _NC_CACHE
    if _NC_CACHE is None:
        _NC_CACHE = build_nc()
    return _NC_CACHE


def _wo_perm(w_out):
    # chunk order [AG0: r0..r3 -> w_out rows 256r..256r+128,
    #              AG1: r0..r3 -> w_out rows 256r+128..256r+256]
    blocks = [w_out[256 * r : 256 * r + 128] for r in range(4)]
    blocks += [w_out[256 * r + 128 : 256 * r + 256] for r in range(4)]
    return np.concatenate(blocks, axis=0)


def _make_in_maps(x, w_qkv, w_out, b_out):
    wop = _wo_perm(w_out)
    in_maps = []
    for c in range(CORES):
        bi = c // GROUP_SIZE
        g = c % GROUP_SIZE
        cols = slice(g * CS, (g + 1) * CS)
        in_maps.append(
            {
                "xt": np.ascontiguousarray(x[bi].T).astype(NP_BF16),
                "wq": np.ascontiguousarray(w_qkv[:, cols]).astype(NP_BF16),
                "wk": np.ascontiguousarray(w_qkv[:, INNER:][:, cols]).astype(NP_BF16),
                "wv": np.ascontiguousarray(w_qkv[:, 2 * INNER:][:, cols]).astype(
                    NP_BF16
                ),
                "wo": np.ascontiguousarray(wop[:, cols]).astype(NP_BF16),
                "bo": np.ascontiguousarray(b_out[cols]),
            }
        )
    return in_maps


def _assemble(results):
    out = np.empty((B, N, DIM), dtype=np.float32)
    for c in range(CORES):
        bi = c // GROUP_SIZE
        g = c % GROUP_SIZE
        out[bi, :, g * CS : (g + 1) * CS] = results[c]["y"].T
    return out


def kernel(x, w_qkv, w_out, b_out, _trace=False, _trace_kwargs=None):
    x = np.asarray(x, dtype=np.float32)
    w_qkv = np.asarray(w_qkv, dtype=np.float32)
    w_out = np.asarray(w_out, dtype=np.float32)
    b_out = np.asarray(b_out, dtype=np.float32)
    nc = _get_nc()
    in_maps = _make_in_maps(x, w_qkv, w_out, b_out)
    res = run_bass_kernel_spmd(
        nc,
        in_maps,
        core_ids=list(range(CORES)),
        trace=_trace,
        **(_trace_kwargs or {}),
    )
    out = _assemble(res.results)
    if _trace:
        return out, res
    return out
